# revision 14
# baseline (speedup 1.0000x reference)
"""Trainium2 Bass kernel for nn_CNN_55422257988257 (CRNN: CNN + FC stack +
4-layer LSTM encoder + 256-step autoregressive 4-layer LSTM decoder).

8 NeuronCores, SPMD:
  - CNN replicated per core (fp32r matmuls on the PE, direct strided-window
    rhs APs, no im2col materialization).
  - FC1 (51200->2048, 420 MB of weights - the memory-bound phase) is
    tensor-parallel by output across the 8 cores (256 outputs each), joined
    by one AllGather. Everything after is replicated (no more collectives).
  - Decoder: wavefront over (step, layer). Each slot computes up to 4
    independent cells placed at PSUM partition bases {0,32,64,96} via PE
    column-group tiling (bf16 weights, concurrent rhs streams). LSTM
    pointwise runs per cell on ACT/DVE; the new h is transposed back into
    lhsT columns on the PE (tail elements + bias folded into zero-padded
    weight chunks).
"""
import os
import sys

sys.path.insert(0, "/opt/trn_rl_repo")

import numpy as np

HID = 257
NG = 4 * HID          # 1028
LAYERS = 4
N_CORES = 8
T_STEPS = int(os.environ.get("KRN_STEPS", "256"))
DEBUG_OUT = os.environ.get("KRN_DEBUG", "0") == "1"

CONV_SPECS = [(64, 32, 3), (128, 64, 3), (256, 128, 3), (512, 256, 3),
              (512, 512, 3), (512, 512, 2)]
CONV_IN = [32, 30, 28, 26, 24, 22]
CONV_OUT = [30, 28, 26, 24, 22, 21]

# free-axis chunk strides with one row of slack (conv6 output: 21 rows x 22)
FIN = [s * s + s for s in CONV_IN]          # input chunk stride per layer
FOUT = [s * s + s for s in CONV_OUT[:5]] + [21 * 22]

_PROGRAM_CACHE = {}


def _gate_reorder(w):
    """torch gate order (i,f,g,o) -> ours (f,i,o,g) along axis 0."""
    i, f, g, o = np.split(np.asarray(w, np.float32), 4, axis=0)
    return np.concatenate([f, i, o, g], axis=0)


def build_program():
    import concourse.bacc as bacc
    import concourse.mybir as mybir
    import concourse.tile as tile

    F32 = mybir.dt.float32
    F32R = mybir.dt.float32r
    BF16 = mybir.dt.bfloat16
    AF = mybir.ActivationFunctionType
    ALU = mybir.AluOpType

    nc = bacc.Bacc("TRN2", target_bir_lowering=False, debug=False,
                   num_devices=N_CORES)

    dram = {}

    def din(name, shape, dt):
        dram[name] = nc.dram_tensor(name, list(shape), dt,
                                    kind="ExternalInput").ap()
        return dram[name]

    din("src", [32, 64 * 64], F32R)
    for li, (co, ci, kk) in enumerate(CONV_SPECS):
        kc = (ci + 127) // 128
        mc = (co + 127) // 128
        mw = min(co, 128)
        din(f"cw{li}", [kc * 128, mc * kk * kk * mw], F32R)
        din(f"cb{li}", [128, mc], F32)
    din("w1", [400 * 128, 256], F32R)
    din("b1", [1, 256], F32R)
    din("w2", [16 * 128, 512], F32R)
    din("b2", [1, 512], F32R)
    din("w3", [4 * 128, 128], F32R)
    din("b3", [1, 128], F32R)
    din("ew0", [2 * 128, NG], BF16)
    for li in range(1, LAYERS):
        din(f"ew{li}", [3 * 128, NG], BF16)
    din("dw", [LAYERS * 6 * 128, NG], BF16)
    din("ones_f", [128, 1], F32)
    din("ones_r", [128, 1], F32R)

    out_y = nc.dram_tensor("y", [max(T_STEPS, 1), HID], F32,
                           kind="ExternalOutput").ap()
    if DEBUG_OUT:
        dbg = nc.dram_tensor("dbg", [8, 2048], F32, kind="ExternalOutput").ap()

    with tile.TileContext(nc) as tc:
        with tc.tile_pool(name="pers", bufs=1) as pers, \
             tc.tile_pool(name="stream", bufs=1) as stream, \
             tc.tile_pool(name="convw", bufs=2) as convwp, \
             tc.tile_pool(name="wstream", bufs=6) as wstream, \
             tc.tile_pool(name="dramp", bufs=1, space="DRAM") as dramp:

            ones_f = pers.tile([128, 1], F32, tag="ones_f")
            nc.sync.dma_start(ones_f[:], dram["ones_f"][:])
            ones_r = pers.tile([128, 1], F32R, tag="ones_r")
            nc.sync.dma_start(ones_r[:], dram["ones_r"][:])

            # ================= CNN =================
            def v3(t, a, b):
                return t.rearrange("p (a b) -> p a b", a=a, b=b)

            xa = stream.tile([32, FIN[0]], F32R, tag="acta")
            with tc.tile_pool(name="srcp", bufs=1) as srcp:
                x0 = srcp.tile([32, 64 * 64], F32R, tag="src")
                nc.sync.dma_start(x0[:], dram["src"][:])
                tpool0 = srcp.tile([32, 32 * 32], F32R, tag="tp0")
                x0v = v3(x0[:], 64, 64)
                nc.vector.tensor_tensor(v3(tpool0[:], 32, 32),
                                        x0v[:, 0:64:2, 0:64:2],
                                        x0v[:, 0:64:2, 1:64:2], ALU.max)
                nc.vector.tensor_tensor(v3(xa[:, 0:1024], 32, 32),
                                        x0v[:, 1:64:2, 0:64:2],
                                        x0v[:, 1:64:2, 1:64:2], ALU.max)
                nc.vector.tensor_tensor(
                    v3(xa[:, 0:1024], 32, 32), v3(xa[:, 0:1024], 32, 32),
                    v3(tpool0[:], 32, 32), ALU.max)

            cur = xa
            tags = ["actb", "acta"]
            with tc.tile_pool(name="psA", bufs=8, space="PSUM") as psA:
                for li, (co, ci, kk) in enumerate(CONV_SPECS):
                    kc = (ci + 127) // 128
                    mc = (co + 127) // 128
                    mw = min(co, 128)
                    si, so = CONV_IN[li], CONV_OUT[li]
                    fin, fout = FIN[li], FOUT[li]

                    bt = stream.tile([128, mc], F32, tag=f"cb{li}")
                    nc.sync.dma_start(bt[:], dram[f"cb{li}"][:])
                    out_t = stream.tile([128, mc * fout], F32R, tag=tags[li % 2])

                    if li == 5:
                        nsplit = [(0, 462)]
                        ocols = 22
                    else:
                        nsplit = [(0, so * so // 2), (so * so // 2, so * so)] \
                            if so * so > 512 else [(0, so * so)]
                        ocols = so

                    gtiles = {}
                    for mi in range(mc):
                        for (n0, n1) in nsplit:
                            gtiles[(mi, n0)] = psA.tile([128, 512], F32, tag="convps", name=f"cps{li}_{mi}_{n0}")

                    for cc in range(kc):
                        ksz = min(ci - cc * 128, 128)
                        wt = convwp.tile([128, mc * kk * kk * mw], F32R, tag="convw", name=f"cw{li}_{cc}")
                        nc.sync.dma_start(wt[:],
                                          dram[f"cw{li}"][128 * cc:128 * (cc + 1), :])
                        for mi in range(mc):
                            mpart = min(co - mi * 128, 128)
                            for (n0, n1) in nsplit:
                                g = gtiles[(mi, n0)]
                                nrow0 = n0 // ocols
                                nrows = (n1 - n0) // ocols
                                for kh in range(kk):
                                    for kw_ in range(kk):
                                        off = cc * fin + (nrow0 + kh) * si + kw_
                                        base = cur[0:ksz, off:off + nrows * si]
                                        rv = v3(base, nrows, si)[:, :, 0:ocols]
                                        nc.tensor.matmul(
                                            g[0:mpart, 0:n1 - n0],
                                            wt[:ksz, ((mi * kk + kh) * kk + kw_) * mw:
                                               ((mi * kk + kh) * kk + kw_) * mw + mpart],
                                            rv,
                                            start=(cc == 0 and kh == 0 and kw_ == 0),
                                            stop=(cc == kc - 1 and kh == kk - 1
                                                  and kw_ == kk - 1))
                    for mi in range(mc):
                        mpart = min(co - mi * 128, 128)
                        for (n0, n1) in nsplit:
                            nc.scalar.activation(
                                out_t[0:mpart, mi * fout + n0:mi * fout + n1],
                                gtiles[(mi, n0)][0:mpart, 0:n1 - n0], AF.Relu,
                                bias=bt[0:mpart, mi:mi + 1])
                    cur = out_t

            # final maxpool 3x3 stride 2: (21x21 in 21x22 layout) -> 10x10
            feat = pers.tile([128, 4 * 100], F32R, tag="feat")
            x6v = cur[:].rearrange("p (c a b) -> p c a b", c=4, a=21, b=22)
            first = True
            for kh in range(3):
                for kw_ in range(3):
                    vtap = x6v[:, :, kh:kh + 19:2, kw_:kw_ + 19:2]
                    fv = feat[:].rearrange("p (c a b) -> p c a b", c=4, a=10)
                    if first:
                        nc.vector.tensor_copy(fv, vtap)
                        first = False
                    else:
                        nc.vector.tensor_tensor(fv, fv, vtap, ALU.max)

            # ================= FC1 (TP-8 by output) =================
            psB_cm = tc.tile_pool(name="psB", bufs=1, space="PSUM")
            psB = psB_cm.__enter__()
            fc1_ps = psB.tile([1, 512], F32, tag="fc1ps")
            for j in range(400):
                wb = wstream.tile([128, 256], F32R, tag="w1buf", name=f"w1b{j}", bufs=4)
                nc.sync.dma_start(wb[:], dram["w1"][128 * j:128 * (j + 1), :])
                s, cch = j // 4, j % 4
                nc.tensor.matmul(fc1_ps[0:1, 0:256],
                                 feat[:, cch * 100 + s:cch * 100 + s + 1],
                                 wb[:], start=(j == 0), stop=False)
            b1t = stream.tile([1, 256], F32R, tag="b1t")
            nc.sync.dma_start(b1t[:], dram["b1"][:])
            nc.tensor.matmul(fc1_ps[0:1, 0:256], ones_r[0:1, :], b1t[:],
                             start=False, stop=True)
            x1s = stream.tile([1, 256], F32, tag="x1s")
            nc.scalar.activation(x1s[:], fc1_ps[0:1, 0:256], AF.Relu)

            ag_in = dramp.tile([1, 256], F32, tag="agin")
            ag_out = dramp.tile([N_CORES, 256], F32, tag="agout")
            nc.sync.dma_start(ag_in[:], x1s[:])
            nc.gpsimd.collective_compute(
                "AllGather", ALU.bypass,
                replica_groups=[list(range(N_CORES))],
                ins=[ag_in.opt()], outs=[ag_out.opt()])
            x1 = stream.tile([1, 2048], F32, tag="x1")
            nc.sync.dma_start(x1[:], ag_out[:].rearrange("r f -> (r f)").unsqueeze(0))

            # ================= FC2 / FC3 =================
            def row_to_cols(row_t, ncols, tag):
                """fp32 row [1, 128*ncols] @p0 -> f32r sbuf cols [128, ncols]."""
                tp = psB.tile([128, ncols], F32, tag="rowcolps")
                for j in range(ncols):
                    nc.tensor.matmul(tp[:, j:j + 1],
                                     row_t[0:1, 128 * j:128 * (j + 1)],
                                     ones_f[0:1, 0:1], is_transpose=True)
                sb_t = pers.tile([128, ncols], F32R, tag=tag)
                nc.vector.tensor_copy(sb_t[:], tp[:])
                return sb_t

            x1c = row_to_cols(x1, 16, "x1c")
            fc2_ps = psB.tile([1, 512], F32, tag="fc2ps")
            for j in range(16):
                wb = wstream.tile([128, 512], F32R, tag="w2buf", name=f"w2b{j}", bufs=2)
                nc.sync.dma_start(wb[:], dram["w2"][128 * j:128 * (j + 1), :])
                nc.tensor.matmul(fc2_ps[0:1, :], x1c[:, j:j + 1], wb[:],
                                 start=(j == 0), stop=False)
            b2t = stream.tile([1, 512], F32R, tag="b2t")
            nc.sync.dma_start(b2t[:], dram["b2"][:])
            nc.tensor.matmul(fc2_ps[0:1, :], ones_r[0:1, :], b2t[:],
                             start=False, stop=True)
            x2 = stream.tile([1, 512], F32, tag="x2")
            nc.scalar.activation(x2[:], fc2_ps[0:1, :], AF.Relu)

            x2c = row_to_cols(x2, 4, "x2c")
            w3t = stream.tile([128, 4 * 128], F32R, tag="w3t")
            nc.sync.dma_start(
                w3t[:].rearrange("p (k w) -> p k w", k=4),
                dram["w3"][:].rearrange("(k p) w -> p k w", p=128))
            fc3_ps = psB.tile([1, 128], F32, tag="fc3ps")
            for j in range(4):
                nc.tensor.matmul(fc3_ps[0:1, :], x2c[:, j:j + 1],
                                 w3t[:, 128 * j:128 * (j + 1)],
                                 start=(j == 0), stop=False)
            b3t = stream.tile([1, 128], F32R, tag="b3t")
            nc.sync.dma_start(b3t[:], dram["b3"][:])
            nc.tensor.matmul(fc3_ps[0:1, :], ones_r[0:1, :], b3t[:],
                             start=False, stop=True)
            x3 = stream.tile([1, 128], F32, tag="x3")
            nc.vector.tensor_copy(x3[:], fc3_ps[0:1, :])

            # ================= LSTM state =================
            # per-layer h rows [1, 384] @p0: [h(257) | 1.0 | zeros]
            h_l = []
            for l in range(LAYERS):
                t_ = pers.tile([1, 384], F32, tag=f"hl{l}", name=f"hl{l}")
                nc.vector.memset(t_[:], 0.0)
                nc.vector.tensor_copy(t_[:, 257:258], ones_f[0:1, :])
                h_l.append(t_)
            # per-layer transposed-h bf16 columns [128, 3] (+1.0 pad in row 1 of col 2)
            xT_l = []
            for l in range(LAYERS):
                t_ = pers.tile([128, 3], BF16, tag=f"xTl{l}", name=f"xTl{l}")
                nc.vector.memset(t_[:], 0.0)
                xT_l.append(t_)
            # per-layer [c | tanh(g)] rows
            cc_l = []
            for l in range(LAYERS):
                t_ = pers.tile([1, 2 * HID], F32, tag=f"ccl{l}", name=f"ccl{l}")
                nc.vector.memset(t_[:], 0.0)
                cc_l.append(t_)
            sg_t = pers.tile([1, 3 * HID], F32, tag="sg")
            mm_t = pers.tile([1, 2 * HID], F32, tag="mm")
            tct_t = pers.tile([1, HID], F32, tag="tct")

            # encoder weights (bf16, same chunk scheme as decoder x-part):
            # layer0: 2 chunks [x0, xtail]; layers1-3: 3 chunks [x0, x1, xtail]
            enc_w = []
            for li in range(LAYERS):
                rr = 2 if li == 0 else 3
                t_ = stream.tile([128, 3 * NG], BF16, tag="encw", name=f"encw{li}")
                nc.sync.dma_start(
                    t_[:, 0:rr * NG].rearrange("p (k w) -> p k w", k=rr),
                    dram[f"ew{li}"][:].rearrange("(k p) w -> p k w", p=128))
                enc_w.append(t_)
            dw_t = pers.tile([128, LAYERS * 6 * NG], BF16, tag="dw")
            nc.sync.dma_start(
                dw_t[:].rearrange("p (c w) -> p c w", c=LAYERS * 6),
                dram["dw"][:].rearrange("(c p) w -> p c w", p=128))

            def dwv(l, q):
                return dw_t[:, ((l * 6 + q) * NG):((l * 6 + q) * NG + NG)]

            # x3 -> padded row [1, 384] then bf16 columns [128, 2]
            x3r = pers.tile([1, 384], F32, tag="x3r")
            nc.vector.memset(x3r[:], 0.0)
            nc.vector.tensor_copy(x3r[:, 257:258], ones_f[0:1, :])
            nc.vector.tensor_copy(x3r[:, 0:128], x3[:])
            x3c = pers.tile([128, 2], BF16, tag="x3c")
            x3ps = psB.tile([128, 2], F32, tag="rowcolps", name="x3ps")
            nc.tensor.matmul(x3ps[:, 0:1], x3r[0:1, 0:128], ones_f[0:1, 0:1],
                             is_transpose=True)
            nc.tensor.matmul(x3ps[:, 1:2], x3r[0:1, 256:384], ones_f[0:1, 0:1],
                             is_transpose=True)
            nc.vector.tensor_copy(x3c[:], x3ps[:])
            psB_cm.__exit__(None, None, None)
            psC_cm = tc.tile_pool(name="psC", bufs=2, space="PSUM")
            psC = psC_cm.__enter__()

            NSPLIT = [(0, 512), (512, 1024), (1024, NG)]

            def cell(kind, l, t):
                """One LSTM cell at partition base 0.

                kind: 'enc' or 'dec'. Returns after writing h_l[l], xT_l[l],
                cc_l[l]."""
                g_ps = psC.tile([1, 1536], F32, tag="gps", name=f"g{kind}{l}_{t}")
                if kind == "enc":
                    nr = 2 if l == 0 else 3
                    srcs = [(x3c if l == 0 else xT_l[l - 1], q) for q in range(nr)]
                    wsrc = [enc_w[l][:, q * NG:(q + 1) * NG] for q in range(nr)]
                else:
                    # h-part rounds first (prefetchable), then x-part
                    srcs = [(xT_l[l], 0), (xT_l[l], 1), (xT_l[l], 2)]
                    wsrc = [dwv(l, 0), dwv(l, 1), dwv(l, 2)]
                    if not (t == 0 and l == 0):
                        xs = xT_l[(l - 1) % 4]
                        srcs += [(xs, 0), (xs, 1), (xs, 2)]
                        wsrc += [dwv(l, 3), dwv(l, 4), dwv(l, 5)]
                nr = len(srcs)
                for ri in range(nr):
                    st, col = srcs[ri]
                    for (n0, n1) in NSPLIT:
                        nc.tensor.matmul(g_ps[0:1, n0:n1], st[:, col:col + 1],
                                         wsrc[ri][:, n0:n1],
                                         start=(ri == 0), stop=(ri == nr - 1))
                # pointwise, gate order (f, i, o, g)
                nc.scalar.activation(cc_l[l][0:1, HID:2 * HID],
                                     g_ps[0:1, 3 * HID:4 * HID], AF.Tanh)
                nc.scalar.activation(sg_t[0:1, :], g_ps[0:1, 0:3 * HID], AF.Sigmoid)
                if kind == "enc":
                    nc.vector.tensor_tensor(cc_l[l][0:1, 0:HID],
                                            sg_t[0:1, HID:2 * HID],
                                            cc_l[l][0:1, HID:2 * HID], ALU.mult)
                else:
                    nc.vector.tensor_tensor(mm_t[0:1, :], sg_t[0:1, 0:2 * HID],
                                            cc_l[l][0:1, 0:2 * HID], ALU.mult)
                    nc.vector.tensor_tensor(cc_l[l][0:1, 0:HID], mm_t[0:1, 0:HID],
                                            mm_t[0:1, HID:2 * HID], ALU.add)
                nc.scalar.activation(tct_t[0:1, :], cc_l[l][0:1, 0:HID], AF.Tanh)
                nc.vector.tensor_tensor(h_l[l][0:1, 0:HID],
                                        sg_t[0:1, 2 * HID:3 * HID],
                                        tct_t[0:1, :], ALU.mult)
                tp = psC.tile([128, 3], F32, tag="htp", name=f"tp{kind}{l}_{t}")
                for q in range(3):
                    nc.tensor.matmul(tp[:, q:q + 1],
                                     h_l[l][0:1, 128 * q:128 * (q + 1)],
                                     ones_f[0:1, 0:1], is_transpose=True)
                nc.vector.tensor_copy(xT_l[l][:], tp[:])

            # ================= encoder =================
            for l in range(LAYERS):
                cell("enc", l, 0)

            # ================= decoder (strictly sequential cells) ==========
            for t in range(T_STEPS):
                for l in range(LAYERS):
                    cell("dec", l, t)
                nc.sync.dma_start(out_y[t:t + 1, :], h_l[3][0:1, 0:HID])

            psC_cm.__exit__(None, None, None)
            if DEBUG_OUT:
                nc.sync.dma_start(dbg[0:1, 0:2048], x1[:])
                nc.sync.dma_start(dbg[1:2, 0:512], x2[:])
                nc.sync.dma_start(dbg[2:3, 0:128], x3[:])
                nc.sync.dma_start(dbg[3:4, 0:400], feat[0:1, :].bitcast(F32))
                for l in range(LAYERS):
                    nc.sync.dma_start(dbg[4 + l:5 + l, 0:HID],
                                      h_l[l][0:1, 0:HID])

    nc.compile()
    return nc


def prep_inputs(src, params):
    import ml_dtypes
    p = params
    base = {}
    base["src"] = np.ascontiguousarray(
        np.asarray(src, np.float32).reshape(32, 64 * 64))
    for li, (co, ci, kk) in enumerate(CONV_SPECS):
        kc = (ci + 127) // 128
        mc = (co + 127) // 128
        mw = min(co, 128)
        w = np.asarray(p["conv_w"][li], np.float32)
        arr = np.zeros((kc, 128, mc, kk, kk, mw), np.float32)
        for cch in range(kc):
            ksz = min(ci - cch * 128, 128)
            for mi in range(mc):
                msz = min(co - mi * 128, 128)
                arr[cch, :ksz, mi, :, :, :msz] = np.transpose(
                    w[mi * 128:mi * 128 + msz, cch * 128:cch * 128 + ksz],
                    (1, 2, 3, 0))
        base[f"cw{li}"] = np.ascontiguousarray(
            arr.reshape(kc * 128, mc * kk * kk * mw))
        b = np.asarray(p["conv_b"][li], np.float32)
        bb = np.zeros((128, mc), np.float32)
        for mi in range(mc):
            msz = min(co - mi * 128, 128)
            bb[:msz, mi] = b[mi * 128:mi * 128 + msz]
        base[f"cb{li}"] = bb

    w1 = np.asarray(p["fc_w"][0], np.float32)       # [2048, 51200]
    b1 = np.asarray(p["fc_b"][0], np.float32)
    w1r = np.ascontiguousarray(
        w1.reshape(2048, 512, 100).transpose(2, 1, 0).reshape(400 * 128, 2048))
    base["w2"] = np.ascontiguousarray(np.asarray(p["fc_w"][1], np.float32).T)
    base["b2"] = np.asarray(p["fc_b"][1], np.float32).reshape(1, 512)
    base["w3"] = np.ascontiguousarray(np.asarray(p["fc_w"][2], np.float32).T)
    base["b3"] = np.asarray(p["fc_b"][2], np.float32).reshape(1, 128)

    enc = p["enc"]
    for li in range(LAYERS):
        Wih = _gate_reorder(enc[li]["Wih"])
        bias = _gate_reorder(np.asarray(enc[li]["bih"], np.float32).reshape(-1, 1)
                             + np.asarray(enc[li]["bhh"], np.float32).reshape(-1, 1))[:, 0]
        if li == 0:
            arr = np.zeros((2, 128, NG), np.float32)
            arr[0] = Wih.T[0:128]
            arr[1, 1] = bias          # pairs with the 1.0 pad (row 1 of xtail col)
            base["ew0"] = arr.reshape(2 * 128, NG).astype(ml_dtypes.bfloat16)
        else:
            arr = np.zeros((3, 128, NG), np.float32)
            arr[0] = Wih.T[0:128]
            arr[1] = Wih.T[128:256]
            arr[2, 0] = Wih.T[256]
            arr[2, 1] = bias
            base[f"ew{li}"] = arr.reshape(3 * 128, NG).astype(ml_dtypes.bfloat16)

    dec = p["dec"]
    dwa = np.zeros((LAYERS, 6, 128, NG), np.float32)
    for li in range(LAYERS):
        Wih = _gate_reorder(dec[li]["Wih"])
        Whh = _gate_reorder(dec[li]["Whh"])
        bias = _gate_reorder(np.asarray(dec[li]["bih"], np.float32).reshape(-1, 1)
                             + np.asarray(dec[li]["bhh"], np.float32).reshape(-1, 1))[:, 0]
        dwa[li, 0] = Whh.T[0:128]
        dwa[li, 1] = Whh.T[128:256]
        dwa[li, 2, 0] = Whh.T[256]
        dwa[li, 2, 1] = bias
        dwa[li, 3] = Wih.T[0:128]
        dwa[li, 4] = Wih.T[128:256]
        dwa[li, 5, 0] = Wih.T[256]
    base["dw"] = dwa.reshape(LAYERS * 6 * 128, NG).astype(ml_dtypes.bfloat16)

    base["ones_f"] = np.ones((128, 1), np.float32)
    base["ones_r"] = np.ones((128, 1), np.float32)

    in_maps = []
    for r in range(N_CORES):
        m = dict(base)
        m["w1"] = np.ascontiguousarray(w1r[:, 256 * r:256 * (r + 1)])
        m["b1"] = np.ascontiguousarray(b1[256 * r:256 * (r + 1)]).reshape(1, 256)
        in_maps.append(m)
    return in_maps


def run(src, params, trace=False):
    from concourse.bass_utils import run_bass_kernel_spmd
    if "prog" not in _PROGRAM_CACHE:
        _PROGRAM_CACHE["prog"] = build_program()
    nc = _PROGRAM_CACHE["prog"]
    in_maps = prep_inputs(np.asarray(src).reshape(32, 64, 64), params)
    res = run_bass_kernel_spmd(nc, in_maps, core_ids=list(range(N_CORES)),
                               trace=trace)
    return res


def kernel(src, params, maxlinelen):
    res = run(src, params)
    return res.results[0]["y"]


# revision 15
# speedup vs baseline: 1.0137x; 1.0137x over previous
"""Trainium2 Bass kernel for nn_CNN_55422257988257 (CRNN: CNN + FC stack +
4-layer LSTM encoder + 256-step autoregressive 4-layer LSTM decoder).

8 NeuronCores, SPMD:
  - CNN replicated per core (fp32r matmuls on the PE, direct strided-window
    rhs APs, no im2col materialization).
  - FC1 (51200->2048, 420 MB of weights - the memory-bound phase) is
    tensor-parallel by output across the 8 cores (256 outputs each), joined
    by one AllGather. Everything after is replicated (no more collectives).
  - Decoder: wavefront over (step, layer). Each slot computes up to 4
    independent cells placed at PSUM partition bases {0,32,64,96} via PE
    column-group tiling (bf16 weights, concurrent rhs streams). LSTM
    pointwise runs per cell on ACT/DVE; the new h is transposed back into
    lhsT columns on the PE (tail elements + bias folded into zero-padded
    weight chunks).
"""
import os
import sys

sys.path.insert(0, "/opt/trn_rl_repo")

import numpy as np

HID = 257
NG = 4 * HID          # 1028
LAYERS = 4
N_CORES = 8
T_STEPS = int(os.environ.get("KRN_STEPS", "256"))
DEBUG_OUT = os.environ.get("KRN_DEBUG", "0") == "1"
NO_CC = os.environ.get("KRN_NO_CC", "0") == "1"

CONV_SPECS = [(64, 32, 3), (128, 64, 3), (256, 128, 3), (512, 256, 3),
              (512, 512, 3), (512, 512, 2)]
CONV_IN = [32, 30, 28, 26, 24, 22]
CONV_OUT = [30, 28, 26, 24, 22, 21]

# free-axis chunk strides with one row of slack (conv6 output: 21 rows x 22)
FIN = [s * s + s for s in CONV_IN]          # input chunk stride per layer
FOUT = [s * s + s for s in CONV_OUT[:5]] + [21 * 22]

_PROGRAM_CACHE = {}


def _gate_reorder(w):
    """torch gate order (i,f,g,o) -> ours (f,i,o,g) along axis 0."""
    i, f, g, o = np.split(np.asarray(w, np.float32), 4, axis=0)
    return np.concatenate([f, i, o, g], axis=0)


def build_program():
    import concourse.bacc as bacc
    import concourse.mybir as mybir
    import concourse.tile as tile

    F32 = mybir.dt.float32
    F32R = mybir.dt.float32r
    BF16 = mybir.dt.bfloat16
    AF = mybir.ActivationFunctionType
    ALU = mybir.AluOpType

    nc = bacc.Bacc("TRN2", target_bir_lowering=False, debug=False,
                   num_devices=N_CORES)

    dram = {}

    def din(name, shape, dt):
        dram[name] = nc.dram_tensor(name, list(shape), dt,
                                    kind="ExternalInput").ap()
        return dram[name]

    din("src", [32, 64 * 64], F32R)
    for li, (co, ci, kk) in enumerate(CONV_SPECS):
        kc = (ci + 127) // 128
        mc = (co + 127) // 128
        mw = min(co, 128)
        din(f"cw{li}", [kc * 128, mc * kk * kk * mw], F32R)
        din(f"cb{li}", [128, mc], F32)
    din("w1", [400 * 128, 256], F32R)
    din("b1", [1, 256], F32R)
    din("w2", [16 * 128, 512], F32R)
    din("b2", [1, 512], F32R)
    din("w3", [4 * 128, 128], F32R)
    din("b3", [1, 128], F32R)
    din("ew0", [2 * 128, NG], BF16)
    for li in range(1, LAYERS):
        din(f"ew{li}", [3 * 128, NG], BF16)
    din("dw", [LAYERS * 6 * 128, NG], BF16)
    din("ones_f", [128, 1], F32)
    din("ones_r", [128, 1], F32R)

    out_y = nc.dram_tensor("y", [max(T_STEPS, 1), HID], F32,
                           kind="ExternalOutput").ap()
    if DEBUG_OUT:
        dbg = nc.dram_tensor("dbg", [8, 2048], F32, kind="ExternalOutput").ap()

    with tile.TileContext(nc) as tc:
        with tc.tile_pool(name="pers", bufs=1) as pers, \
             tc.tile_pool(name="stream", bufs=1) as stream, \
             tc.tile_pool(name="convw", bufs=2) as convwp, \
             tc.tile_pool(name="wstream", bufs=6) as wstream, \
             tc.tile_pool(name="dramp", bufs=1, space="DRAM") as dramp:

            ones_f = pers.tile([128, 1], F32, tag="ones_f")
            nc.sync.dma_start(ones_f[:], dram["ones_f"][:])
            ones_r = pers.tile([128, 1], F32R, tag="ones_r")
            nc.sync.dma_start(ones_r[:], dram["ones_r"][:])

            # ================= CNN =================
            def v3(t, a, b):
                return t.rearrange("p (a b) -> p a b", a=a, b=b)

            xa = stream.tile([32, FIN[0]], F32R, tag="acta")
            with tc.tile_pool(name="srcp", bufs=1) as srcp:
                x0 = srcp.tile([32, 64 * 64], F32R, tag="src")
                nc.sync.dma_start(x0[:], dram["src"][:])
                tpool0 = srcp.tile([32, 32 * 32], F32R, tag="tp0")
                x0v = v3(x0[:], 64, 64)
                nc.vector.tensor_tensor(v3(tpool0[:], 32, 32),
                                        x0v[:, 0:64:2, 0:64:2],
                                        x0v[:, 0:64:2, 1:64:2], ALU.max)
                nc.vector.tensor_tensor(v3(xa[:, 0:1024], 32, 32),
                                        x0v[:, 1:64:2, 0:64:2],
                                        x0v[:, 1:64:2, 1:64:2], ALU.max)
                nc.vector.tensor_tensor(
                    v3(xa[:, 0:1024], 32, 32), v3(xa[:, 0:1024], 32, 32),
                    v3(tpool0[:], 32, 32), ALU.max)

            cur = xa
            tags = ["actb", "acta"]
            with tc.tile_pool(name="psA", bufs=8, space="PSUM") as psA:
                for li, (co, ci, kk) in enumerate(CONV_SPECS):
                    kc = (ci + 127) // 128
                    mc = (co + 127) // 128
                    mw = min(co, 128)
                    si, so = CONV_IN[li], CONV_OUT[li]
                    fin, fout = FIN[li], FOUT[li]

                    bt = stream.tile([128, mc], F32, tag=f"cb{li}")
                    nc.sync.dma_start(bt[:], dram[f"cb{li}"][:])
                    out_t = stream.tile([128, mc * fout], F32R, tag=tags[li % 2])

                    if li == 5:
                        nsplit = [(0, 462)]
                        ocols = 22
                    else:
                        nsplit = [(0, so * so // 2), (so * so // 2, so * so)] \
                            if so * so > 512 else [(0, so * so)]
                        ocols = so

                    gtiles = {}
                    for mi in range(mc):
                        for (n0, n1) in nsplit:
                            gtiles[(mi, n0)] = psA.tile([128, 512], F32, tag="convps", name=f"cps{li}_{mi}_{n0}")

                    for cc in range(kc):
                        ksz = min(ci - cc * 128, 128)
                        wt = convwp.tile([128, mc * kk * kk * mw], F32R, tag="convw", name=f"cw{li}_{cc}")
                        nc.sync.dma_start(wt[:],
                                          dram[f"cw{li}"][128 * cc:128 * (cc + 1), :])
                        for mi in range(mc):
                            mpart = min(co - mi * 128, 128)
                            for (n0, n1) in nsplit:
                                g = gtiles[(mi, n0)]
                                nrow0 = n0 // ocols
                                nrows = (n1 - n0) // ocols
                                for kh in range(kk):
                                    for kw_ in range(kk):
                                        off = cc * fin + (nrow0 + kh) * si + kw_
                                        base = cur[0:ksz, off:off + nrows * si]
                                        rv = v3(base, nrows, si)[:, :, 0:ocols]
                                        nc.tensor.matmul(
                                            g[0:mpart, 0:n1 - n0],
                                            wt[:ksz, ((mi * kk + kh) * kk + kw_) * mw:
                                               ((mi * kk + kh) * kk + kw_) * mw + mpart],
                                            rv,
                                            start=(cc == 0 and kh == 0 and kw_ == 0),
                                            stop=(cc == kc - 1 and kh == kk - 1
                                                  and kw_ == kk - 1))
                    for mi in range(mc):
                        mpart = min(co - mi * 128, 128)
                        for (n0, n1) in nsplit:
                            nc.scalar.activation(
                                out_t[0:mpart, mi * fout + n0:mi * fout + n1],
                                gtiles[(mi, n0)][0:mpart, 0:n1 - n0], AF.Relu,
                                bias=bt[0:mpart, mi:mi + 1])
                    cur = out_t

            # final maxpool 3x3 stride 2: (21x21 in 21x22 layout) -> 10x10
            feat = pers.tile([128, 4 * 100], F32R, tag="feat")
            x6v = cur[:].rearrange("p (c a b) -> p c a b", c=4, a=21, b=22)
            first = True
            for kh in range(3):
                for kw_ in range(3):
                    vtap = x6v[:, :, kh:kh + 19:2, kw_:kw_ + 19:2]
                    fv = feat[:].rearrange("p (c a b) -> p c a b", c=4, a=10)
                    if first:
                        nc.vector.tensor_copy(fv, vtap)
                        first = False
                    else:
                        nc.vector.tensor_tensor(fv, fv, vtap, ALU.max)

            # ================= FC1 (TP-8 by output) =================
            psB_cm = tc.tile_pool(name="psB", bufs=1, space="PSUM")
            psB = psB_cm.__enter__()
            fc1_ps = psB.tile([1, 512], F32, tag="fc1ps")
            for j in range(400):
                wb = wstream.tile([128, 256], F32R, tag="w1buf", name=f"w1b{j}", bufs=4)
                nc.sync.dma_start(wb[:], dram["w1"][128 * j:128 * (j + 1), :])
                s, cch = j // 4, j % 4
                nc.tensor.matmul(fc1_ps[0:1, 0:256],
                                 feat[:, cch * 100 + s:cch * 100 + s + 1],
                                 wb[:], start=(j == 0), stop=False)
            b1t = stream.tile([1, 256], F32R, tag="b1t")
            nc.sync.dma_start(b1t[:], dram["b1"][:])
            nc.tensor.matmul(fc1_ps[0:1, 0:256], ones_r[0:1, :], b1t[:],
                             start=False, stop=True)
            x1s = stream.tile([1, 256], F32, tag="x1s")
            nc.scalar.activation(x1s[:], fc1_ps[0:1, 0:256], AF.Relu)

            ag_in = dramp.tile([1, 256], F32, tag="agin")
            ag_out = dramp.tile([N_CORES, 256], F32, tag="agout")
            nc.sync.dma_start(ag_in[:], x1s[:])
            if NO_CC:
                for _r in range(N_CORES):
                    nc.sync.dma_start(ag_out[_r:_r + 1, :], ag_in[:])
            else:
                nc.gpsimd.collective_compute(
                    "AllGather", ALU.bypass,
                    replica_groups=[list(range(N_CORES))],
                    ins=[ag_in.opt()], outs=[ag_out.opt()])
            x1 = stream.tile([1, 2048], F32, tag="x1")
            nc.sync.dma_start(x1[:], ag_out[:].rearrange("r f -> (r f)").unsqueeze(0))

            # ================= FC2 / FC3 =================
            def row_to_cols(row_t, ncols, tag):
                """fp32 row [1, 128*ncols] @p0 -> f32r sbuf cols [128, ncols]."""
                tp = psB.tile([128, ncols], F32, tag="rowcolps")
                for j in range(ncols):
                    nc.tensor.matmul(tp[:, j:j + 1],
                                     row_t[0:1, 128 * j:128 * (j + 1)],
                                     ones_f[0:1, 0:1], is_transpose=True)
                sb_t = pers.tile([128, ncols], F32R, tag=tag)
                nc.vector.tensor_copy(sb_t[:], tp[:])
                return sb_t

            x1c = row_to_cols(x1, 16, "x1c")
            fc2_ps = psB.tile([1, 512], F32, tag="fc2ps")
            for j in range(16):
                wb = wstream.tile([128, 512], F32R, tag="w2buf", name=f"w2b{j}", bufs=2)
                nc.sync.dma_start(wb[:], dram["w2"][128 * j:128 * (j + 1), :])
                nc.tensor.matmul(fc2_ps[0:1, :], x1c[:, j:j + 1], wb[:],
                                 start=(j == 0), stop=False)
            b2t = stream.tile([1, 512], F32R, tag="b2t")
            nc.sync.dma_start(b2t[:], dram["b2"][:])
            nc.tensor.matmul(fc2_ps[0:1, :], ones_r[0:1, :], b2t[:],
                             start=False, stop=True)
            x2 = stream.tile([1, 512], F32, tag="x2")
            nc.scalar.activation(x2[:], fc2_ps[0:1, :], AF.Relu)

            x2c = row_to_cols(x2, 4, "x2c")
            w3t = stream.tile([128, 4 * 128], F32R, tag="w3t")
            nc.sync.dma_start(
                w3t[:].rearrange("p (k w) -> p k w", k=4),
                dram["w3"][:].rearrange("(k p) w -> p k w", p=128))
            fc3_ps = psB.tile([1, 128], F32, tag="fc3ps")
            for j in range(4):
                nc.tensor.matmul(fc3_ps[0:1, :], x2c[:, j:j + 1],
                                 w3t[:, 128 * j:128 * (j + 1)],
                                 start=(j == 0), stop=False)
            b3t = stream.tile([1, 128], F32R, tag="b3t")
            nc.sync.dma_start(b3t[:], dram["b3"][:])
            nc.tensor.matmul(fc3_ps[0:1, :], ones_r[0:1, :], b3t[:],
                             start=False, stop=True)
            x3 = stream.tile([1, 128], F32, tag="x3")
            nc.vector.tensor_copy(x3[:], fc3_ps[0:1, :])

            # ================= LSTM state =================
            # per-layer h rows [1, 384] @p0: [h(257) | 1.0 | zeros]
            h_l = []
            for l in range(LAYERS):
                t_ = pers.tile([1, 384], F32, tag=f"hl{l}", name=f"hl{l}")
                nc.vector.memset(t_[:], 0.0)
                nc.vector.tensor_copy(t_[:, 257:258], ones_f[0:1, :])
                h_l.append(t_)
            # per-layer transposed-h bf16 columns [128, 3] (+1.0 pad in row 1 of col 2)
            xT_l = []
            for l in range(LAYERS):
                t_ = pers.tile([128, 3], BF16, tag=f"xTl{l}", name=f"xTl{l}")
                nc.vector.memset(t_[:], 0.0)
                xT_l.append(t_)
            # per-layer [c | tanh(g)] rows
            cc_l = []
            for l in range(LAYERS):
                t_ = pers.tile([1, 2 * HID], F32, tag=f"ccl{l}", name=f"ccl{l}")
                nc.vector.memset(t_[:], 0.0)
                cc_l.append(t_)
            sg_t = pers.tile([1, 3 * HID], F32, tag="sg")
            mm_t = pers.tile([1, 2 * HID], F32, tag="mm")
            tct_t = pers.tile([1, HID], F32, tag="tct")

            # encoder weights (bf16, same chunk scheme as decoder x-part):
            # layer0: 2 chunks [x0, xtail]; layers1-3: 3 chunks [x0, x1, xtail]
            enc_w = []
            for li in range(LAYERS):
                rr = 2 if li == 0 else 3
                t_ = stream.tile([128, 3 * NG], BF16, tag="encw", name=f"encw{li}")
                nc.sync.dma_start(
                    t_[:, 0:rr * NG].rearrange("p (k w) -> p k w", k=rr),
                    dram[f"ew{li}"][:].rearrange("(k p) w -> p k w", p=128))
                enc_w.append(t_)
            dw_t = pers.tile([128, LAYERS * 6 * NG], BF16, tag="dw")
            nc.sync.dma_start(
                dw_t[:].rearrange("p (c w) -> p c w", c=LAYERS * 6),
                dram["dw"][:].rearrange("(c p) w -> p c w", p=128))

            def dwv(l, q):
                return dw_t[:, ((l * 6 + q) * NG):((l * 6 + q) * NG + NG)]

            # x3 -> padded row [1, 384] then bf16 columns [128, 2]
            x3r = pers.tile([1, 384], F32, tag="x3r")
            nc.vector.memset(x3r[:], 0.0)
            nc.vector.tensor_copy(x3r[:, 257:258], ones_f[0:1, :])
            nc.vector.tensor_copy(x3r[:, 0:128], x3[:])
            x3c = pers.tile([128, 2], BF16, tag="x3c")
            x3ps = psB.tile([128, 2], F32, tag="rowcolps", name="x3ps")
            nc.tensor.matmul(x3ps[:, 0:1], x3r[0:1, 0:128], ones_f[0:1, 0:1],
                             is_transpose=True)
            nc.tensor.matmul(x3ps[:, 1:2], x3r[0:1, 256:384], ones_f[0:1, 0:1],
                             is_transpose=True)
            nc.vector.tensor_copy(x3c[:], x3ps[:])
            psB_cm.__exit__(None, None, None)
            psC_cm = tc.tile_pool(name="psC", bufs=2, space="PSUM")
            psC = psC_cm.__enter__()

            NSPLIT = [(0, 512), (512, 1024), (1024, NG)]

            def cell(kind, l, t):
                """One LSTM cell at partition base 0.

                kind: 'enc' or 'dec'. Returns after writing h_l[l], xT_l[l],
                cc_l[l]."""
                g_ps = psC.tile([1, 1536], F32, tag="gps", name=f"g{kind}{l}_{t}")
                if kind == "enc":
                    nr = 2 if l == 0 else 3
                    srcs = [(x3c if l == 0 else xT_l[l - 1], q) for q in range(nr)]
                    wsrc = [enc_w[l][:, q * NG:(q + 1) * NG] for q in range(nr)]
                else:
                    # h-part rounds first (prefetchable), then x-part
                    srcs = [(xT_l[l], 0), (xT_l[l], 1), (xT_l[l], 2)]
                    wsrc = [dwv(l, 0), dwv(l, 1), dwv(l, 2)]
                    if not (t == 0 and l == 0):
                        xs = xT_l[(l - 1) % 4]
                        srcs += [(xs, 0), (xs, 1), (xs, 2)]
                        wsrc += [dwv(l, 3), dwv(l, 4), dwv(l, 5)]
                nr = len(srcs)
                for ri in range(nr):
                    st, col = srcs[ri]
                    for (n0, n1) in NSPLIT:
                        nc.tensor.matmul(g_ps[0:1, n0:n1], st[:, col:col + 1],
                                         wsrc[ri][:, n0:n1],
                                         start=(ri == 0), stop=(ri == nr - 1))
                # pointwise, gate order (f, i, o, g)
                nc.scalar.activation(cc_l[l][0:1, HID:2 * HID],
                                     g_ps[0:1, 3 * HID:4 * HID], AF.Tanh)
                nc.scalar.activation(sg_t[0:1, :], g_ps[0:1, 0:3 * HID], AF.Sigmoid)
                if kind == "enc":
                    nc.vector.tensor_tensor(cc_l[l][0:1, 0:HID],
                                            sg_t[0:1, HID:2 * HID],
                                            cc_l[l][0:1, HID:2 * HID], ALU.mult)
                else:
                    nc.vector.tensor_tensor(mm_t[0:1, :], sg_t[0:1, 0:2 * HID],
                                            cc_l[l][0:1, 0:2 * HID], ALU.mult)
                    nc.vector.tensor_tensor(cc_l[l][0:1, 0:HID], mm_t[0:1, 0:HID],
                                            mm_t[0:1, HID:2 * HID], ALU.add)
                nc.scalar.activation(tct_t[0:1, :], cc_l[l][0:1, 0:HID], AF.Tanh)
                nc.vector.tensor_tensor(h_l[l][0:1, 0:HID],
                                        sg_t[0:1, 2 * HID:3 * HID],
                                        tct_t[0:1, :], ALU.mult)
                tp = psC.tile([128, 3], F32, tag="htp", name=f"tp{kind}{l}_{t}")
                for q in range(3):
                    nc.tensor.matmul(tp[:, q:q + 1],
                                     h_l[l][0:1, 128 * q:128 * (q + 1)],
                                     ones_f[0:1, 0:1], is_transpose=True)
                nc.vector.tensor_copy(xT_l[l][:], tp[:])

            # ================= encoder =================
            for l in range(LAYERS):
                cell("enc", l, 0)

            # ================= decoder (strictly sequential cells) ==========
            for t in range(T_STEPS):
                for l in range(LAYERS):
                    cell("dec", l, t)
                nc.sync.dma_start(out_y[t:t + 1, :], h_l[3][0:1, 0:HID])

            psC_cm.__exit__(None, None, None)
            if DEBUG_OUT:
                nc.sync.dma_start(dbg[0:1, 0:2048], x1[:])
                nc.sync.dma_start(dbg[1:2, 0:512], x2[:])
                nc.sync.dma_start(dbg[2:3, 0:128], x3[:])
                nc.sync.dma_start(dbg[3:4, 0:400], feat[0:1, :].bitcast(F32))
                for l in range(LAYERS):
                    nc.sync.dma_start(dbg[4 + l:5 + l, 0:HID],
                                      h_l[l][0:1, 0:HID])

    nc.compile()
    return nc


def prep_inputs(src, params):
    import ml_dtypes
    p = params
    base = {}
    base["src"] = np.ascontiguousarray(
        np.asarray(src, np.float32).reshape(32, 64 * 64))
    for li, (co, ci, kk) in enumerate(CONV_SPECS):
        kc = (ci + 127) // 128
        mc = (co + 127) // 128
        mw = min(co, 128)
        w = np.asarray(p["conv_w"][li], np.float32)
        arr = np.zeros((kc, 128, mc, kk, kk, mw), np.float32)
        for cch in range(kc):
            ksz = min(ci - cch * 128, 128)
            for mi in range(mc):
                msz = min(co - mi * 128, 128)
                arr[cch, :ksz, mi, :, :, :msz] = np.transpose(
                    w[mi * 128:mi * 128 + msz, cch * 128:cch * 128 + ksz],
                    (1, 2, 3, 0))
        base[f"cw{li}"] = np.ascontiguousarray(
            arr.reshape(kc * 128, mc * kk * kk * mw))
        b = np.asarray(p["conv_b"][li], np.float32)
        bb = np.zeros((128, mc), np.float32)
        for mi in range(mc):
            msz = min(co - mi * 128, 128)
            bb[:msz, mi] = b[mi * 128:mi * 128 + msz]
        base[f"cb{li}"] = bb

    w1 = np.asarray(p["fc_w"][0], np.float32)       # [2048, 51200]
    b1 = np.asarray(p["fc_b"][0], np.float32)
    w1r = np.ascontiguousarray(
        w1.reshape(2048, 512, 100).transpose(2, 1, 0).reshape(400 * 128, 2048))
    base["w2"] = np.ascontiguousarray(np.asarray(p["fc_w"][1], np.float32).T)
    base["b2"] = np.asarray(p["fc_b"][1], np.float32).reshape(1, 512)
    base["w3"] = np.ascontiguousarray(np.asarray(p["fc_w"][2], np.float32).T)
    base["b3"] = np.asarray(p["fc_b"][2], np.float32).reshape(1, 128)

    enc = p["enc"]
    for li in range(LAYERS):
        Wih = _gate_reorder(enc[li]["Wih"])
        bias = _gate_reorder(np.asarray(enc[li]["bih"], np.float32).reshape(-1, 1)
                             + np.asarray(enc[li]["bhh"], np.float32).reshape(-1, 1))[:, 0]
        if li == 0:
            arr = np.zeros((2, 128, NG), np.float32)
            arr[0] = Wih.T[0:128]
            arr[1, 1] = bias          # pairs with the 1.0 pad (row 1 of xtail col)
            base["ew0"] = arr.reshape(2 * 128, NG).astype(ml_dtypes.bfloat16)
        else:
            arr = np.zeros((3, 128, NG), np.float32)
            arr[0] = Wih.T[0:128]
            arr[1] = Wih.T[128:256]
            arr[2, 0] = Wih.T[256]
            arr[2, 1] = bias
            base[f"ew{li}"] = arr.reshape(3 * 128, NG).astype(ml_dtypes.bfloat16)

    dec = p["dec"]
    dwa = np.zeros((LAYERS, 6, 128, NG), np.float32)
    for li in range(LAYERS):
        Wih = _gate_reorder(dec[li]["Wih"])
        Whh = _gate_reorder(dec[li]["Whh"])
        bias = _gate_reorder(np.asarray(dec[li]["bih"], np.float32).reshape(-1, 1)
                             + np.asarray(dec[li]["bhh"], np.float32).reshape(-1, 1))[:, 0]
        dwa[li, 0] = Whh.T[0:128]
        dwa[li, 1] = Whh.T[128:256]
        dwa[li, 2, 0] = Whh.T[256]
        dwa[li, 2, 1] = bias
        dwa[li, 3] = Wih.T[0:128]
        dwa[li, 4] = Wih.T[128:256]
        dwa[li, 5, 0] = Wih.T[256]
    base["dw"] = dwa.reshape(LAYERS * 6 * 128, NG).astype(ml_dtypes.bfloat16)

    base["ones_f"] = np.ones((128, 1), np.float32)
    base["ones_r"] = np.ones((128, 1), np.float32)

    in_maps = []
    for r in range(N_CORES):
        m = dict(base)
        m["w1"] = np.ascontiguousarray(w1r[:, 256 * r:256 * (r + 1)])
        m["b1"] = np.ascontiguousarray(b1[256 * r:256 * (r + 1)]).reshape(1, 256)
        in_maps.append(m)
    return in_maps


def run(src, params, trace=False):
    from concourse.bass_utils import run_bass_kernel_spmd
    if "prog" not in _PROGRAM_CACHE:
        _PROGRAM_CACHE["prog"] = build_program()
    nc = _PROGRAM_CACHE["prog"]
    in_maps = prep_inputs(np.asarray(src).reshape(32, 64, 64), params)
    res = run_bass_kernel_spmd(nc, in_maps, core_ids=list(range(N_CORES)),
                               trace=trace)
    return res


def kernel(src, params, maxlinelen):
    res = run(src, params)
    return res.results[0]["y"]
